# revision 14
# baseline (speedup 1.0000x reference)
"""GAT (2-layer) + global attention pooling + MLP classifier on 8 Trainium2 cores.

Strategy (sharding_hint: shard nodes + incoming edges across devices):
  - dst nodes sharded contiguously: core c owns nodes [c*6250, (c+1)*6250).
  - Edges assigned to the core owning their dst, sorted by dst; packed on host
    into "groups" of <=128 edges covering <=32 whole dst segments (a dst's
    in-edges never split across groups -> each dst is reduced in exactly one
    group, so scatters are plain writes, no read-modify-write).
  - Per group the device: indirect-DMA gathers source-node feature rows (by
    src) and attention scalars (el by src, er by dst), builds a slot-selector
    matrix from host-provided local-slot ids with one is_equal, multiplies in
    the edge softmax weights exp(leaky_relu(el+er)) (softmax max-subtraction
    dropped - mathematically identical, values are O(1)), and uses PE matmuls
    to segment-reduce edges -> dst slots, fusing the softmax denominator as an
    extra matmul column. The per-head fc weight is applied *after* aggregation
    (matmul commutes with the weighted sum), avoiding any [N,H*D] feature
    table materialization for layer 1.
  - Inter-layer: h1 shards + layer-2 attention scalars are AllGathered to a
    [N,68] table that layer 2 gathers from.
  - Readout: per-graph softmax pooling via onehot(graph)*exp(gate) matmuls
    accumulated over the shard, AllReduce of the [65,8] partial (numerator
    rows + denominator row), classifier replicated on every core.
  - Small GAT/MLP weights replicated to all cores (host-precomputed fold:
    vl=W_fc@attn_l per head).
"""

import numpy as np

# Problem constants (fixed by the reference).
N = 50000
E = 500000
G = 8
IN = 128
HID = 64
H = 2
C = 2
SLOPE = np.float32(0.2)
NCORES = 8
SH = N // NCORES  # 6250 nodes per core

P = 128      # edges per group (partition dim)
SLOTS = 32   # dst slots per group
KG = 4       # groups per supertile
PAD_LDST = np.float32(99.0)
DEN_EPS = np.float32(1e-30)

_f32 = np.float32


# --------------------------------------------------------------------------
# Host-side edge packing
# --------------------------------------------------------------------------

def _pack_edges(src, dst):
    """Pack edges into per-core supertile index arrays.

    Returns dict with per-core arrays (lists indexed by core):
      srct  [T, P, KG] int32   source node id per edge slot (pad -> 0)
      dstet [T, P, KG] int32   global dst node id per edge slot (pad -> 0)
      ldst  [T, P, KG] float32 slot id within group 0..31 (pad -> 99)
      slot  [T, P] int32       local dst row (0..SH-1) per slot (pad -> SH)
      T     supertile count (same for every core)
    """
    src = np.asarray(src).astype(np.int64)
    dst = np.asarray(dst).astype(np.int64)
    order = np.argsort(dst, kind="stable")
    ssrc, sdst = src[order], dst[order]
    # rowptr over all N dsts
    counts = np.bincount(sdst, minlength=N)
    rowptr = np.zeros(N + 1, dtype=np.int64)
    np.cumsum(counts, out=rowptr[1:])

    cores = []
    max_groups = 0
    for c in range(NCORES):
        lo_node, hi_node = c * SH, (c + 1) * SH
        groups = []  # (esrc, edst, eldst, slots)
        g_src, g_dst, g_ldst, g_slots = [], [], [], []
        for d in range(lo_node, hi_node):
            e0, e1 = rowptr[d], rowptr[d + 1]
            seg = e1 - e0
            if len(g_src) + seg > P or len(g_slots) == SLOTS:
                groups.append((g_src, g_dst, g_ldst, g_slots))
                g_src, g_dst, g_ldst, g_slots = [], [], [], []
            s = len(g_slots)
            g_slots.append(d - lo_node)
            if seg:
                g_src.extend(ssrc[e0:e1])
                g_dst.extend([d] * seg)
                g_ldst.extend([s] * seg)
        groups.append((g_src, g_dst, g_ldst, g_slots))
        cores.append(groups)
        max_groups = max(max_groups, len(groups))

    T = -(-max_groups // KG)  # ceil div
    out = {"T": T, "srct": [], "dstet": [], "ldst": [], "slot": []}
    for c in range(NCORES):
        groups = cores[c]
        srct = np.zeros((T, P, KG), dtype=np.int32)
        dstet = np.zeros((T, P, KG), dtype=np.int32)
        ldst = np.full((T, P, KG), PAD_LDST, dtype=np.float32)
        slot = np.full((T, P), SH, dtype=np.int32)
        for gi, (g_src, g_dst, g_ldst, g_slots) in enumerate(groups):
            t, g = divmod(gi, KG)
            ne = len(g_src)
            srct[t, :ne, g] = g_src
            dstet[t, :ne, g] = g_dst
            ldst[t, :ne, g] = g_ldst
            ns = len(g_slots)
            slot[t, g * SLOTS:g * SLOTS + ns] = g_slots
        out["srct"].append(srct)
        out["dstet"].append(dstet)
        out["ldst"].append(ldst)
        out["slot"].append(slot)
    return out


def _fold_weights(W_fc, attn_l, attn_r, F):
    """vl[h] = W_fc[:, h] @ attn_l[h]  -> Vcat [F, 4] cols (el0, el1, er0, er1)."""
    Wr = W_fc.reshape(F, H, HID)
    vcat = np.zeros((F, 4), dtype=np.float32)
    for h in range(H):
        vcat[:, h] = (Wr[:, h, :] @ attn_l[h]).astype(np.float32)
        vcat[:, 2 + h] = (Wr[:, h, :] @ attn_r[h]).astype(np.float32)
    return vcat


# --------------------------------------------------------------------------
# Numpy mirror of the device program (for validation / debugging)
# --------------------------------------------------------------------------

def _np_layer(pack, table, eler_full, W_fc, F):
    """Run one GAT layer exactly the way the device does.

    table     [N, F]  gather source rows (features per node)
    eler_full [N, 4]  (el_h0, el_h1, er_h0, er_h1) per node
    Returns h_out [N, HID] (relu + head-mean applied).
    """
    Wr = W_fc.reshape(F, H, HID).astype(_f32)
    h_out = np.zeros((N, HID), dtype=_f32)
    iota = np.arange(SLOTS, dtype=_f32)
    for c in range(NCORES):
        T = pack["T"]
        srct, dstet = pack["srct"][c], pack["dstet"][c]
        ldstt, slott = pack["ldst"][c], pack["slot"][c]
        hloc = np.zeros((SH + 1, HID), dtype=_f32)
        for t in range(T):
            for g in range(KG):
                esrc = srct[t, :, g]
                edst = dstet[t, :, g]
                ldst = ldstt[t, :, g]
                slots = slott[t, g * SLOTS:(g + 1) * SLOTS]
                yraw = table[esrc].astype(_f32)            # [P, F]
                el = eler_full[esrc, :2].astype(_f32)      # [P, 2]
                er = eler_full[edst, 2:4].astype(_f32)     # [P, 2]
                e = el + er
                e = np.where(e > 0, e, SLOPE * e).astype(_f32)
                w = np.exp(e).astype(_f32)                 # [P, 2]
                sel = (ldst[:, None] == iota[None, :]).astype(_f32)  # [P, 32]
                for h in range(H):
                    selw = sel * w[:, h:h + 1]             # [P, 32]
                    A = yraw.T.astype(_f32) @ selw         # [F, 32]
                    den = selw.sum(axis=0).astype(_f32)    # [32]
                    B = Wr[:, h, :].T @ A                  # [HID, 32]
                    den = np.maximum(den, DEN_EPS)
                    out = (B / den[None, :]).T             # [32, HID]
                    out = np.maximum(out, 0.0)
                    if h == 0:
                        acc = out
                    else:
                        acc = ((acc + out) * _f32(0.5)).astype(_f32)
                hloc[slots] = acc
        h_out[c * SH:(c + 1) * SH] = hloc[:SH]
    return h_out


def _np_forward(pack, inputs):
    h_n = np.asarray(inputs["h_n"], dtype=_f32)
    v1 = _fold_weights(np.asarray(inputs["W_fc1"], _f32),
                       np.asarray(inputs["attn_l1"], _f32),
                       np.asarray(inputs["attn_r1"], _f32), IN)
    v2 = _fold_weights(np.asarray(inputs["W_fc2"], _f32),
                       np.asarray(inputs["attn_l2"], _f32),
                       np.asarray(inputs["attn_r2"], _f32), HID)
    eler1 = (h_n @ v1).astype(_f32)
    h1 = _np_layer(pack, h_n, eler1, np.asarray(inputs["W_fc1"], _f32), IN)
    eler2 = (h1 @ v2).astype(_f32)
    h2 = _np_layer(pack, h1, eler2, np.asarray(inputs["W_fc2"], _f32), HID)

    gid = np.asarray(inputs["graph_ids"]).astype(np.int64)
    w_gate = np.asarray(inputs["w_gate"], _f32)
    b_gate = np.asarray(inputs["b_gate"], _f32)
    gate = (h2 @ w_gate[:, 0] + b_gate[0]).astype(_f32)    # [N]
    aexp = np.exp(gate).astype(_f32)
    oh = (gid[:, None] == np.arange(G)[None, :]).astype(_f32)  # [N, 8]
    num = np.zeros((HID + 1, G), dtype=_f32)
    num[:HID] = h2.T @ (oh * aexp[:, None])
    num[HID] = aexp @ oh
    den = np.maximum(num[HID], DEN_EPS)
    hg = (num[:HID] / den[None, :]).T                      # [G, HID]
    a = (aexp / den[gid]).astype(_f32)[:, None]            # [N, 1]
    W_c1 = np.asarray(inputs["W_c1"], _f32)
    b_c1 = np.asarray(inputs["b_c1"], _f32)
    W_c2 = np.asarray(inputs["W_c2"], _f32)
    b_c2 = np.asarray(inputs["b_c2"], _f32)
    a2 = (hg @ W_c1 + b_c1).astype(_f32)
    a3 = (a2 @ W_c2 + b_c2).astype(_f32)
    sig = (1.0 / (1.0 + np.exp(-a3))).astype(_f32)
    return sig, a, hg.astype(_f32)


# --------------------------------------------------------------------------
# Bass program
# --------------------------------------------------------------------------

def _build_program(T):
    import concourse.bacc as bacc
    import concourse.mybir as mybir
    import concourse.tile as tile
    from concourse.bass import IndirectOffsetOnAxis

    dt = mybir.dt
    F32, I32 = dt.float32, dt.int32
    Alu = mybir.AluOpType
    Act = mybir.ActivationFunctionType
    RG = [list(range(NCORES))]
    NT = (SH + P - 1) // P  # node tiles per shard
    HC = HID + 1            # transposed column count (feat + den)

    nc = bacc.Bacc("TRN2", target_bir_lowering=False, debug=False,
                   num_devices=NCORES)

    def din(name, shape, dtype=F32):
        return nc.dram_tensor(name, list(shape), dtype, kind="ExternalInput").ap()

    h_n = din("h_n", (N, IN))
    hshard = din("h_shard", (SH, IN))
    srct = din("srct", (T, P, KG), I32)
    dstet = din("dstet", (T, P, KG), I32)
    ldstt = din("ldstt", (T, P, KG))
    slott = din("slott", (T, P), I32)
    gidf = din("gidf", (SH, 1))
    iota32 = din("iota32", (P, KG * SLOTS))
    iota8 = din("iota8", (P, G))
    ones128 = din("ones128", (P, 1))
    onesrow = din("onesrow", (1, P))
    ident = din("ident", (P, P))
    v1rep = din("v1rep", (P, 4 * IN))
    v2rep = din("v2rep", (P, 4 * HID))
    wgrep = din("wgrep", (P, HID))
    bgate = din("bgate", (P, 1))
    w1h = din("w1h", (IN, H * HID))
    w2h = din("w2h", (HID, H * HID))
    wc1 = din("wc1", (HID, HID))
    bc1 = din("bc1", (HID, 1))
    wc2 = din("wc2", (HID, C))
    bc2 = din("bc2", (C, 1))

    out_sig = nc.dram_tensor("out_sig", [G, C], F32, kind="ExternalOutput").ap()
    out_a = nc.dram_tensor("out_a", [SH, 1], F32, kind="ExternalOutput").ap()
    out_hg = nc.dram_tensor("out_hg", [G, HID], F32, kind="ExternalOutput").ap()

    h1_shard = nc.dram_tensor("h1_shard", [SH + 1, HID], F32, kind="Internal").ap()
    h2_shard = nc.dram_tensor("h2_shard", [SH + 1, HID], F32, kind="Internal").ap()
    agin1 = nc.dram_tensor("agin1", [SH, 4], F32, kind="Internal").ap()
    eler1 = nc.dram_tensor("eler1", [N, 4], F32, kind="Internal",
                           addr_space="Shared").ap()
    agin2 = nc.dram_tensor("agin2", [SH, HID + 4], F32, kind="Internal").ap()
    table2 = nc.dram_tensor("table2", [N, HID + 4], F32, kind="Internal",
                            addr_space="Shared").ap()
    arin = nc.dram_tensor("arin", [HC, G], F32, kind="Internal").ap()
    arout = nc.dram_tensor("arout", [HC, G], F32, kind="Internal",
                           addr_space="Shared").ap()

    with tile.TileContext(nc) as tc, \
         tc.tile_pool(name="consts", bufs=1) as cp:

        def const(ap_, shape, dtype=F32):
            t_ = cp.tile(list(shape), dtype, name=f"c_{ap_.tensor.name}")
            nc.sync.dma_start(t_[:], ap_)
            return t_

        c_iota32 = const(iota32, (P, KG * SLOTS))
        c_iota8 = const(iota8, (P, G))
        c_ones128 = const(ones128, (P, 1))
        c_onesrow = const(onesrow, (1, P))
        c_ident = const(ident, (P, P))
        c_v1rep = const(v1rep, (P, 4 * IN))
        c_v2rep = const(v2rep, (P, 4 * HID))
        c_wgrep = const(wgrep, (P, HID))
        c_bgate = const(bgate, (P, 1))
        c_w1h = const(w1h, (IN, H * HID))
        c_w2h = const(w2h, (HID, H * HID))
        c_wc1 = const(wc1, (HID, HID))
        c_bc1 = const(bc1, (HID, 1))
        c_wc2 = const(wc2, (HID, C))
        c_bc2 = const(bc2, (C, 1))

        # ---- phase A: eler1 = h_shard @ V1cat, allgathered ----
        with tc.tile_pool(name="phA", bufs=3) as ap_:
            for t in range(NT):
                p = min(P, SH - t * P)
                ht = ap_.tile([P, IN], F32, tag="ht")
                nc.sync.dma_start(ht[:p], hshard[t * P:t * P + p, :])
                scr = ap_.tile([P, IN], F32, tag="scr")
                elr = ap_.tile([P, 4], F32, tag="elr")
                for j in range(4):
                    nc.vector.tensor_tensor(
                        out=scr[:p], in0=ht[:p],
                        in1=c_v1rep[:p, j * IN:(j + 1) * IN], op=Alu.mult)
                    nc.vector.reduce_sum(
                        out=elr[:p, j:j + 1], in_=scr[:p],
                        axis=mybir.AxisListType.X)
                nc.sync.dma_start(agin1[t * P:t * P + p, :], elr[:p])
        nc.gpsimd.collective_compute(
            "AllGather", Alu.bypass, replica_groups=RG,
            ins=[agin1.opt()], outs=[eler1.opt()])

        # ---- edge phases (shared emitter for both GAT layers) ----
        def emit_layer(lname, table_ap, F, eler_ap, el_off, er_off, whcat,
                       out_table):
            with tc.tile_pool(name=f"{lname}s", bufs=4) as sp, \
                 tc.tile_pool(name=f"{lname}p", bufs=2, space="PSUM") as pp:
                for t in range(T):
                    idxS = sp.tile([P, KG], I32, tag="idxS")
                    nc.sync.dma_start(idxS[:], srct[t, :, :])
                    idxD = sp.tile([P, KG], I32, tag="idxD")
                    nc.sync.dma_start(idxD[:], dstet[t, :, :])
                    ldt = sp.tile([P, KG], F32, tag="ldt")
                    nc.sync.dma_start(ldt[:], ldstt[t, :, :])
                    slt = sp.tile([P, 1], I32, tag="slt")
                    nc.sync.dma_start(slt[:], slott[t, :, None])

                    # HW indirect DMA consumes exactly one index per
                    # partition (multi-index offset tiles are read as one
                    # row + contiguous overrun), so gathers go per group.
                    yraw = sp.tile([P, KG * F], F32, tag="yraw")
                    elt = sp.tile([P, KG * 2], F32, tag="elt")
                    ert = sp.tile([P, KG * 2], F32, tag="ert")
                    for g in range(KG):
                        nc.gpsimd.indirect_dma_start(
                            out=yraw[:, g * F:(g + 1) * F], out_offset=None,
                            in_=table_ap,
                            in_offset=IndirectOffsetOnAxis(
                                ap=idxS[:, g:g + 1], axis=0))
                        nc.gpsimd.indirect_dma_start(
                            out=elt[:, g * 2:(g + 1) * 2], out_offset=None,
                            in_=eler_ap,
                            in_offset=IndirectOffsetOnAxis(
                                ap=idxS[:, g:g + 1], axis=0),
                            element_offset=el_off)
                        nc.gpsimd.indirect_dma_start(
                            out=ert[:, g * 2:(g + 1) * 2], out_offset=None,
                            in_=eler_ap,
                            in_offset=IndirectOffsetOnAxis(
                                ap=idxD[:, g:g + 1], axis=0),
                            element_offset=er_off)

                    # sel[p, g*32+s] = (ldst[p,g] == s)
                    sel = sp.tile([P, KG * SLOTS], F32, tag="sel")
                    nc.vector.tensor_tensor(
                        out=sel[:].rearrange("p (g s) -> p g s", s=SLOTS),
                        in0=ldt[:, :, None].to_broadcast([P, KG, SLOTS]),
                        in1=c_iota32[:].rearrange("p (g s) -> p g s", s=SLOTS),
                        op=Alu.is_equal)
                    # w = exp(leaky_relu(el + er))  [P, KG*2] cols (g,h)
                    wa = sp.tile([P, KG * 2], F32, tag="wa")
                    nc.vector.tensor_add(out=wa[:], in0=elt[:], in1=ert[:])
                    wb = sp.tile([P, KG * 2], F32, tag="wb")
                    # leaky_relu(x) = max(slope*x, x)
                    nc.vector.scalar_tensor_tensor(
                        out=wb[:], in0=wa[:], scalar=float(SLOPE), in1=wa[:],
                        op0=Alu.mult, op1=Alu.max)
                    wc = sp.tile([P, KG * 2], F32, tag="wc")
                    nc.scalar.activation(out=wc[:], in_=wb[:], func=Act.Exp)
                    # selw cols h*128 + g*32 + s
                    selw = sp.tile([P, 2 * KG * SLOTS], F32, tag="selw")
                    selw4 = selw[:].rearrange("p (h g s) -> p h g s", h=2,
                                              s=SLOTS)
                    wc3 = wc[:].rearrange("p (g h) -> p g h", h=2)
                    sel3 = sel[:].rearrange("p (g s) -> p g s", s=SLOTS)
                    for h in range(H):
                        nc.vector.tensor_tensor(
                            out=selw4[:, h], in0=sel3,
                            in1=wc3[:, :, h:h + 1].to_broadcast(
                                [P, KG, SLOTS]),
                            op=Alu.mult)

                    # A[f, h*128+g*32+s] = sum_e yraw[e, g*F+f] selw[e, ...]
                    aps = pp.tile([F, 2 * KG * SLOTS], F32, tag="aps")
                    for g in range(KG):
                        for h in range(H):
                            co = h * KG * SLOTS + g * SLOTS
                            nc.tensor.matmul(
                                out=aps[:, co:co + SLOTS],
                                lhsT=yraw[:, g * F:(g + 1) * F],
                                rhs=selw[:, co:co + SLOTS],
                                start=True, stop=True)
                    ast = sp.tile([F, 2 * KG * SLOTS], F32, tag="ast")
                    nc.vector.tensor_copy(out=ast[:], in_=aps[:])
                    # B[j, h*128+g*32+s] with den row at partition HID
                    bst = sp.tile([HC, 2 * KG * SLOTS], F32, tag="bst")
                    bps = pp.tile([HC, 2 * KG * SLOTS], F32, tag="bps")
                    for h in range(H):
                        co = h * KG * SLOTS
                        nc.tensor.matmul(
                            out=bps[:HID, co:co + KG * SLOTS],
                            lhsT=whcat[:, h * HID:(h + 1) * HID],
                            rhs=ast[:, co:co + KG * SLOTS],
                            start=True, stop=True)
                    nc.tensor.matmul(out=bps[HID:HC, :], lhsT=c_ones128[:],
                                     rhs=selw[:], start=True, stop=True)
                    nc.vector.tensor_copy(out=bst[:], in_=bps[:])
                    nc.vector.tensor_scalar_max(
                        out=bst[HID:HC, :], in0=bst[HID:HC, :],
                        scalar1=float(DEN_EPS))
                    # transpose to slot-major [128, 65] per head
                    tps = pp.tile([P, 2 * HC], F32, tag="tps")
                    for h in range(H):
                        nc.tensor.transpose(
                            out=tps[:, h * HC:(h + 1) * HC],
                            in_=bst[:, h * KG * SLOTS:(h + 1) * KG * SLOTS],
                            identity=c_ident[:HC, :HC])
                    # epilogue: out = relu(x/den); hm = 0.5*(out0+out1)
                    rd = sp.tile([P, 2], F32, tag="rd")
                    for h in range(H):
                        nc.vector.reciprocal(
                            out=rd[:, h:h + 1],
                            in_=tps[:, h * HC + HID:h * HC + HID + 1])
                    rdh = sp.tile([P, 2], F32, tag="rdh")
                    nc.vector.tensor_scalar_mul(out=rdh[:], in0=rd[:],
                                                scalar1=0.5)
                    r0 = sp.tile([P, HID], F32, tag="r0")
                    r1 = sp.tile([P, HID], F32, tag="r1")
                    for h, dst_t in ((0, r0), (1, r1)):
                        nc.vector.tensor_scalar(
                            out=dst_t[:], in0=tps[:, h * HC:h * HC + HID],
                            scalar1=rdh[:, h:h + 1], scalar2=0.0,
                            op0=Alu.mult, op1=Alu.max)
                    hm = sp.tile([P, HID], F32, tag="hm")
                    nc.vector.tensor_add(out=hm[:], in0=r0[:], in1=r1[:])
                    nc.gpsimd.indirect_dma_start(
                        out=out_table, out_offset=IndirectOffsetOnAxis(
                            ap=slt[:, :1], axis=0),
                        in_=hm[:], in_offset=None)

        emit_layer("L1", h_n, IN, eler1, 0, 2, c_w1h, h1_shard)

        # ---- phase C: agin2 = [h1 | h1 @ V2cat], allgathered to table2 ----
        with tc.tile_pool(name="phC", bufs=3) as cpo:
            for t in range(NT):
                p = min(P, SH - t * P)
                h1t = cpo.tile([P, HID], F32, tag="h1t")
                nc.sync.dma_start(h1t[:p], h1_shard[t * P:t * P + p, :])
                agt = cpo.tile([P, HID + 4], F32, tag="agt")
                nc.vector.tensor_copy(out=agt[:p, :HID], in_=h1t[:p])
                scr2 = cpo.tile([P, HID], F32, tag="scr2")
                for j in range(4):
                    nc.vector.tensor_tensor(
                        out=scr2[:p], in0=h1t[:p],
                        in1=c_v2rep[:p, j * HID:(j + 1) * HID], op=Alu.mult)
                    nc.vector.reduce_sum(
                        out=agt[:p, HID + j:HID + j + 1], in_=scr2[:p],
                        axis=mybir.AxisListType.X)
                nc.sync.dma_start(agin2[t * P:t * P + p, :], agt[:p])
        nc.gpsimd.collective_compute(
            "AllGather", Alu.bypass, replica_groups=RG,
            ins=[agin2.opt()], outs=[table2.opt()])

        emit_layer("L2", table2, HID, table2, HID, HID + 2, c_w2h, h2_shard)

        # ---- phase E: per-graph gate softmax partials ----
        with tc.tile_pool(name="phE", bufs=3) as ep, \
             tc.tile_pool(name="phEr", bufs=1) as er_, \
             tc.tile_pool(name="phEp", bufs=1, space="PSUM") as epp:
            aexp_all = er_.tile([P, NT], F32)
            gid_all = er_.tile([P, NT], F32)
            nps = epp.tile([HC, G], F32)
            for t in range(NT):
                p = min(P, SH - t * P)
                h2t = ep.tile([P, HID], F32, tag="h2t")
                nc.sync.dma_start(h2t[:p], h2_shard[t * P:t * P + p, :])
                nc.sync.dma_start(gid_all[:p, t:t + 1],
                                  gidf[t * P:t * P + p, :])
                h2e = ep.tile([P, HC], F32, tag="h2e")
                nc.vector.tensor_copy(out=h2e[:p, :HID], in_=h2t[:p])
                nc.vector.memset(h2e[:p, HID:HC], 1.0)
                scre = ep.tile([P, HID], F32, tag="scre")
                gt = ep.tile([P, 1], F32, tag="gt")
                nc.vector.tensor_tensor(
                    out=scre[:p], in0=h2t[:p], in1=c_wgrep[:p], op=Alu.mult)
                nc.vector.reduce_sum(out=gt[:p], in_=scre[:p],
                                     axis=mybir.AxisListType.X)
                nc.scalar.activation(out=aexp_all[:p, t:t + 1], in_=gt[:p],
                                     func=Act.Exp, bias=c_bgate[:p])
                oh = ep.tile([P, G], F32, tag="oh")
                nc.vector.tensor_tensor(
                    out=oh[:p], in0=gid_all[:p, t:t + 1].to_broadcast([p, G]),
                    in1=c_iota8[:p], op=Alu.is_equal)
                rhse = ep.tile([P, G], F32, tag="rhse")
                nc.vector.tensor_scalar(
                    out=rhse[:p], in0=oh[:p],
                    scalar1=aexp_all[:p, t:t + 1], scalar2=None,
                    op0=Alu.mult)
                nc.tensor.matmul(out=nps[:], lhsT=h2e[:p], rhs=rhse[:p],
                                 start=(t == 0), stop=(t == NT - 1))
            nums = er_.tile([HC, G], F32)
            nc.vector.tensor_copy(out=nums[:], in_=nps[:])
            nc.sync.dma_start(arin, nums[:])
            nc.gpsimd.collective_compute(
                "AllReduce", Alu.add, replica_groups=RG,
                ins=[arin.opt()], outs=[arout.opt()])

            # ---- phase F: pooled graph vectors + classifier ----
            with tc.tile_pool(name="phF", bufs=1) as fp, \
                 tc.tile_pool(name="phFp", bufs=1, space="PSUM") as fpp:
                numf = fp.tile([HC, G], F32)
                nc.sync.dma_start(numf[:], arout)
                # den row lives at partition HID; DMA it to partition 0
                # (compute engines cannot shift partitions, DMA can)
                denf = fp.tile([1, G], F32)
                nc.sync.dma_start(denf[:], arout[HID:HC, :])
                nc.vector.tensor_scalar_max(out=denf[:], in0=denf[:],
                                            scalar1=float(DEN_EPS))
                rdf = fp.tile([1, G], F32)
                nc.vector.reciprocal(out=rdf[:], in_=denf[:])
                dbps = fpp.tile([P, G], F32)
                nc.tensor.matmul(out=dbps[:], lhsT=c_onesrow[:], rhs=rdf[:],
                                 start=True, stop=True)
                denb = fp.tile([P, G], F32)
                nc.vector.tensor_copy(out=denb[:], in_=dbps[:])
                hgT = fp.tile([HID, G], F32)
                nc.vector.tensor_tensor(out=hgT[:], in0=numf[:HID, :],
                                        in1=denb[:HID, :], op=Alu.mult)
                hgps = fpp.tile([G, HID], F32)
                nc.tensor.transpose(out=hgps[:], in_=hgT[:],
                                    identity=c_ident[:HID, :HID])
                hgsb = fp.tile([G, HID], F32)
                nc.vector.tensor_copy(out=hgsb[:], in_=hgps[:])
                nc.sync.dma_start(out_hg, hgsb[:])
                a2ps = fpp.tile([HID, G], F32)
                nc.tensor.matmul(out=a2ps[:], lhsT=c_wc1[:], rhs=hgT[:],
                                 start=True, stop=True)
                a2sb = fp.tile([HID, G], F32)
                nc.vector.tensor_scalar_add(out=a2sb[:], in0=a2ps[:],
                                            scalar1=c_bc1[:, 0:1])
                a3ps = fpp.tile([C, G], F32)
                nc.tensor.matmul(out=a3ps[:], lhsT=c_wc2[:], rhs=a2sb[:],
                                 start=True, stop=True)
                sigsb = fp.tile([C, G], F32)
                nc.scalar.activation(out=sigsb[:], in_=a3ps[:],
                                     func=Act.Sigmoid, bias=c_bc2[:, 0:1])
                sigps = fpp.tile([G, C], F32)
                nc.tensor.transpose(out=sigps[:], in_=sigsb[:],
                                    identity=c_ident[:C, :C])
                sigout = fp.tile([G, C], F32)
                nc.vector.tensor_copy(out=sigout[:], in_=sigps[:])
                nc.sync.dma_start(out_sig, sigout[:])

                # ---- phase G: per-node attention output a ----
                with tc.tile_pool(name="phG", bufs=3) as gp:
                    for t in range(NT):
                        p = min(P, SH - t * P)
                        oh2 = gp.tile([P, G], F32, tag="oh2")
                        nc.vector.tensor_tensor(
                            out=oh2[:p],
                            in0=gid_all[:p, t:t + 1].to_broadcast([p, G]),
                            in1=c_iota8[:p], op=Alu.is_equal)
                        scrg = gp.tile([P, G], F32, tag="scrg")
                        dg = gp.tile([P, 1], F32, tag="dg")
                        nc.vector.tensor_tensor(
                            out=scrg[:p], in0=oh2[:p], in1=denb[:p],
                            op=Alu.mult)
                        nc.vector.reduce_sum(out=dg[:p], in_=scrg[:p],
                                             axis=mybir.AxisListType.X)
                        at = gp.tile([P, 1], F32, tag="at")
                        nc.vector.tensor_scalar(
                            out=at[:p], in0=aexp_all[:p, t:t + 1],
                            scalar1=dg[:p, 0:1], scalar2=None, op0=Alu.mult)
                        nc.sync.dma_start(out_a[t * P:t * P + p, :], at[:p])

    nc.compile()
    return nc


# --------------------------------------------------------------------------
# Host orchestration
# --------------------------------------------------------------------------

def _make_in_maps(pack, inputs):
    h_n = np.ascontiguousarray(np.asarray(inputs["h_n"], dtype=_f32))
    v1 = _fold_weights(np.asarray(inputs["W_fc1"], _f32),
                       np.asarray(inputs["attn_l1"], _f32),
                       np.asarray(inputs["attn_r1"], _f32), IN)
    v2 = _fold_weights(np.asarray(inputs["W_fc2"], _f32),
                       np.asarray(inputs["attn_l2"], _f32),
                       np.asarray(inputs["attn_r2"], _f32), HID)
    W1 = np.asarray(inputs["W_fc1"], _f32).reshape(IN, H, HID)
    W2 = np.asarray(inputs["W_fc2"], _f32).reshape(HID, H, HID)
    w1h = np.ascontiguousarray(W1.reshape(IN, H * HID))
    w2h = np.ascontiguousarray(W2.reshape(HID, H * HID))
    gid = np.asarray(inputs["graph_ids"]).astype(_f32)[:, None]

    iota32 = np.tile(np.arange(SLOTS, dtype=_f32), KG)[None, :].repeat(P, 0)
    iota8 = np.arange(G, dtype=_f32)[None, :].repeat(P, 0)
    common = {
        "h_n": h_n,
        "iota32": np.ascontiguousarray(iota32),
        "iota8": np.ascontiguousarray(iota8),
        "ones128": np.ones((P, 1), _f32),
        "onesrow": np.ones((1, P), _f32),
        "ident": np.eye(P, dtype=_f32),
        "v1rep": np.ascontiguousarray(v1.T.reshape(1, 4 * IN).repeat(P, 0)),
        "v2rep": np.ascontiguousarray(v2.T.reshape(1, 4 * HID).repeat(P, 0)),
        "wgrep": np.ascontiguousarray(
            np.asarray(inputs["w_gate"], _f32)[:, 0][None, :].repeat(P, 0)),
        "bgate": np.full((P, 1), np.asarray(inputs["b_gate"], _f32)[0], _f32),
        "w1h": w1h,
        "w2h": w2h,
        "wc1": np.ascontiguousarray(np.asarray(inputs["W_c1"], _f32)),
        "bc1": np.ascontiguousarray(np.asarray(inputs["b_c1"], _f32)[:, None]),
        "wc2": np.ascontiguousarray(np.asarray(inputs["W_c2"], _f32)),
        "bc2": np.ascontiguousarray(np.asarray(inputs["b_c2"], _f32)[:, None]),
    }
    in_maps = []
    for c in range(NCORES):
        m = dict(common)
        m["h_shard"] = np.ascontiguousarray(h_n[c * SH:(c + 1) * SH])
        m["srct"] = pack["srct"][c]
        m["dstet"] = pack["dstet"][c]
        m["ldstt"] = pack["ldst"][c]
        m["slott"] = pack["slot"][c]
        m["gidf"] = np.ascontiguousarray(gid[c * SH:(c + 1) * SH])
        in_maps.append(m)
    return in_maps


_PROGRAM_CACHE = {}


def _run(inputs, **spmd_kwargs):
    import sys
    for pth in ("/opt/trn_rl_repo",):
        if pth not in sys.path:
            sys.path.insert(0, pth)
    from concourse.bass_utils import run_bass_kernel_spmd

    pack = _pack_edges(inputs["src"], inputs["dst"])
    T = pack["T"]
    if T not in _PROGRAM_CACHE:
        _PROGRAM_CACHE[T] = _build_program(T)
    nc = _PROGRAM_CACHE[T]
    in_maps = _make_in_maps(pack, inputs)
    res = run_bass_kernel_spmd(nc, in_maps, core_ids=list(range(NCORES)),
                               **spmd_kwargs)
    outs = res.results
    sig = np.asarray(outs[0]["out_sig"], dtype=_f32)
    hg = np.asarray(outs[0]["out_hg"], dtype=_f32)
    a = np.concatenate([np.asarray(outs[c]["out_a"], dtype=_f32)
                        for c in range(NCORES)], axis=0)
    return (sig, a, hg), res


def kernel(**inputs):
    out, _ = _run(inputs)
    return out


def _bench(inputs, iters=20):
    """Steady-state device wall time per kernel execution (ns).

    Rebuilds the same shard_map(jit) that run_bass_via_pjrt uses, but
    without output donation, and with inputs resident on device, so
    repeated calls measure execute latency rather than host transfer.
    """
    import sys
    if "/opt/trn_rl_repo" not in sys.path:
        sys.path.insert(0, "/opt/trn_rl_repo")
    import time
    import jax
    from jax.sharding import Mesh, PartitionSpec, NamedSharding
    from concourse import bass2jax, mybir

    pack = _pack_edges(inputs["src"], inputs["dst"])
    T = pack["T"]
    if T not in _PROGRAM_CACHE:
        _PROGRAM_CACHE[T] = _build_program(T)
    nc = _PROGRAM_CACHE[T]
    in_maps = _make_in_maps(pack, inputs)

    bass2jax.install_neuronx_cc_hook()
    partition_name = (nc.partition_id_tensor.name
                      if nc.partition_id_tensor else None)
    in_names, out_names, out_avals, zero_outs = [], [], [], []
    for alloc in nc.m.functions[0].allocations:
        if not isinstance(alloc, mybir.MemoryLocationSet):
            continue
        name = alloc.memorylocations[0].name
        if alloc.kind == "ExternalInput":
            if name != partition_name:
                in_names.append(name)
        elif alloc.kind == "ExternalOutput":
            out_names.append(name)
            shape = tuple(alloc.tensor_shape)
            dtype = mybir.dt.np(alloc.dtype)
            out_avals.append(jax.core.ShapedArray(shape, dtype))
            zero_outs.append(np.zeros(shape, dtype))
    n_params = len(in_names)
    all_names = list(in_names) + list(out_names)
    if partition_name is not None:
        all_names.append(partition_name)

    def _body(*args):
        operands = list(args)
        if partition_name is not None:
            operands.append(bass2jax.partition_id_tensor())
        outs = bass2jax._bass_exec_p.bind(
            *operands,
            out_avals=tuple(out_avals),
            in_names=tuple(all_names),
            out_names=tuple(out_names),
            lowering_input_output_aliases=(),
            sim_require_finite=True,
            sim_require_nnan=True,
            nc=nc)
        return tuple(outs)

    devices = jax.devices()[:NCORES]
    mesh = Mesh(np.asarray(devices), ("core",))
    nin = n_params + len(zero_outs)
    sharded = jax.jit(bass2jax.shard_map(
        _body, mesh=mesh, in_specs=(PartitionSpec("core"),) * nin,
        out_specs=(PartitionSpec("core"),) * len(out_names),
        check_rep=False), keep_unused=True)

    sh = NamedSharding(mesh, PartitionSpec("core"))
    per_core = [[np.asarray(m[name]) for name in in_names] for m in in_maps]
    dev_in = [jax.device_put(
        np.concatenate([per_core[c][i] for c in range(NCORES)], axis=0), sh)
        for i in range(n_params)]
    dev_zero = [jax.device_put(
        np.zeros((NCORES * z.shape[0], *z.shape[1:]), z.dtype), sh)
        for z in zero_outs]

    out = sharded(*dev_in, *dev_zero)  # compile + warm
    jax.block_until_ready(out)
    times = []
    for _ in range(iters):
        t0 = time.perf_counter()
        out = sharded(*dev_in, *dev_zero)
        jax.block_until_ready(out)
        times.append(time.perf_counter() - t0)
    times.sort()
    best = times[0]
    med = times[len(times) // 2]
    print(f"bench: min {best*1e6:.1f}us  med {med*1e6:.1f}us  "
          f"max {times[-1]*1e6:.1f}us over {iters} iters")
    return best * 1e9
